# revision 10
# baseline (speedup 1.0000x reference)
"""Trainium2 Bass kernel for a 2-layer encoder/decoder LSTM (CS-LSTM).

Model (hardcoded): B=16384, F=2, T_IN=30, T_OUT=50, H=128.
Sharding: pure data parallel over batch across 8 NeuronCores (B_CORE=2048),
weights replicated. No cross-core communication.

Device layout: "hidden on partitions" — h, c, gate tensors are [H=128, B_CORE]
so every recurrent matmul is out[128, N] = W_T.T @ h with K=128 on partitions.

The Activation engine is the bottleneck (~99% busy in the cost-model
timeline): 5 activation instructions per LSTM cell step, each paying a
~185 ns non-pipelineable SBUF/PSUM access-latency overhead on top of the
N/1.2GHz streaming time. This version cuts that to 3 instructions per cell:

  - gates are packed [i, f, o, g] in ONE 4-bank PSUM tile [128, 4*b_half];
    sigmoid over the contiguous [i,f,o] span is a single [128, 3*b_half]
    activation; tanh(g) and tanh(c) stay separate.
  - per-gate biases move out of the activation (the act bias operand is
    per-partition and cannot vary along the free dim): layer-0 cells get the
    bias via a ones-augmented input row (K=2 -> K=3 matmul, free); K=128
    cells get 3 extra K=1 matmuls (bias row x ones) on the under-utilized
    TensorE; tanh(g)'s bias rides the activation bias operand as before.

Other tricks kept from the baseline:
  - x pre-transposed to xT[f, t, b] (+ ones row) so per-step input is a
    cheap contiguous DMA.
  - decoder input projection composed with the fc layer (wcomb = Wih0d@fcW),
    removing the fc matmul from the recurrent critical path.
  - predictions computed with h2-tiles as the stationary operand so they
    land batch-major for a cheap final DMA.
"""

import os

os.environ.setdefault("MYCRO_LOCAL_CACHE", "1")

import numpy as np
from contextlib import ExitStack

import concourse.bass as bass
import concourse.tile as tile
from concourse import bacc, mybir

B, F, T_IN, T_OUT, H = 16384, 2, 30, 50, 128
N_CORES = 8
B_CORE = B // N_CORES

F32 = mybir.dt.float32
F32R = mybir.dt.float32r
AF = mybir.ActivationFunctionType
ALU = mybir.AluOpType

# pytorch gate order is (i, f, g, o); we pack PSUM banks as [i, f, o, g]
# so the three sigmoids are contiguous. GORD[j] = pytorch index of bank j.
GORD = (0, 1, 3, 2)


def build_program(b_core=B_CORE, t_in=T_IN, t_out=T_OUT, n_split=4, n_repeat=1,
                  t1_gpsimd=False, work_bufs=1, h_bufs=2, x_bufs=3):
    """Build + compile the single-core program (same program on all cores)."""
    nc = bacc.Bacc("TRN2", target_bir_lowering=False, debug=False)

    nj = b_core // 128             # number of 128-wide batch tiles (fc)
    b_half = b_core // n_split     # per-chain batch width
    nj_h = b_half // 128
    assert b_half % 128 == 0 and b_half <= 512

    def din(name, shape, dt=F32R):
        return nc.dram_tensor(name, shape, dt, kind="ExternalInput").ap()

    # inputs (all gate-dim blocks packed in GORD order [i, f, o, g])
    xT = din("xT", [F + 1, t_in, b_core])             # x[b,f,t] -> [f,t,b]; row F = ones
    wih0T_e = din("wih0T_e", [F + 1, 4 * H])          # enc_Wih0.T + bias row
    whh0T_e = din("whh0T_e", [H, 4 * H])              # enc_Whh0.T
    wih1T_e = din("wih1T_e", [H, 4 * H])
    whh1T_e = din("whh1T_e", [H, 4 * H])
    wih0T_d = din("wih0T_d", [F + 1, 4 * H])          # dec step-0 input proj + b row
    wcombT = din("wcombT", [H, 4 * H])                # (dec_Wih0@fc_W).T
    whh0T_d = din("whh0T_d", [H, 4 * H])
    wih1T_d = din("wih1T_d", [H, 4 * H])
    whh1T_d = din("whh1T_d", [H, 4 * H])
    brow_e1 = din("brow_e1", [1, 4 * H])              # enc_b1 as a row (GORD)
    brow_comb = din("brow_comb", [1, 4 * H])          # dec_b0 + dec_Wih0@fc_b
    brow_d1 = din("brow_d1", [1, 4 * H])
    ones_r = din("ones_r", [1, b_core])               # rhs for bias matmuls
    bg_e1 = din("bg_e1", [H, 1], F32)                 # gate-g act biases
    bg_comb = din("bg_comb", [H, 1], F32)
    bg_d1 = din("bg_d1", [H, 1], F32)
    fcWT = din("fcWT", [H, F])                        # fc_W.T
    fcb = din("fcb", [H, F * nj], F32)                # fc_b tiled nj times

    out = nc.dram_tensor(
        "out", [b_core, F * t_out], F32, kind="ExternalOutput"
    ).ap()

    with tile.TileContext(nc) as tc:
        with ExitStack() as ctx:
            wpool = ctx.enter_context(tc.tile_pool(name="weights", bufs=1))
            hpool = ctx.enter_context(tc.tile_pool(name="hstate", bufs=h_bufs))
            cpool = ctx.enter_context(tc.tile_pool(name="cstate", bufs=1))
            work = ctx.enter_context(tc.tile_pool(name="work", bufs=work_bufs))
            xpool = ctx.enter_context(tc.tile_pool(name="xin", bufs=x_bufs))
            psum = ctx.enter_context(
                tc.tile_pool(name="psum", bufs=1, space="PSUM")
            )

            def load_w(ap, dt=None):
                t = wpool.tile(list(ap.shape), dt or F32R, tag=f"w_{ap.name}")
                nc.sync.dma_start(t[:], ap[:])
                return t

            w_wih0e = load_w(wih0T_e)
            w_whh0e = load_w(whh0T_e)
            w_wih1e = load_w(wih1T_e)
            w_whh1e = load_w(whh1T_e)
            w_wih0d = load_w(wih0T_d)
            w_wcomb = load_w(wcombT)
            w_whh0d = load_w(whh0T_d)
            w_wih1d = load_w(wih1T_d)
            w_whh1d = load_w(whh1T_d)
            w_fcWT = load_w(fcWT)
            w_be1 = load_w(brow_e1)
            w_bcomb = load_w(brow_comb)
            w_bd1 = load_w(brow_d1)
            w_ones = load_w(ones_r)
            t_bg_e1 = load_w(bg_e1, F32)
            t_bg_comb = load_w(bg_comb, F32)
            t_bg_d1 = load_w(bg_d1, F32)
            t_fcb = load_w(fcb, F32)

            # prediction accumulator: [128, nj*F*t_out], batch tile j occupies
            # free range [j*F*t_out, (j+1)*F*t_out), layout (t, f) within.
            hp = wpool.tile([128, nj * F * t_out], F32, tag="hpred")

            hp3 = hp[:].rearrange("p (j q) -> p j q", q=F * t_out)
            fcb3 = t_fcb[:].rearrange("p (j q) -> p j q", q=F)

            def layer_step(hs, wi, wh, brow, bg, rhs_in, h_t, c_t, htag,
                           first=False):
                """One LSTM cell for one chain.

                brow None  -> bias comes inside wi (augmented input row).
                brow given -> 3 K=1 bias matmuls for i,f,o; bg = [H,1] act
                              bias for tanh(g).
                """
                # g bank first (own 1-bank tile) so tanh(g) overlaps ifo MMs
                psg = psum.tile([H, b_half], F32, tag=f"gg{hs % 2}")
                nc.tensor.matmul(
                    psg[:],
                    wi[:, 128 * 3 : 128 * 4],
                    rhs_in[:],
                    start=True,
                    stop=first,
                )
                if not first:
                    nc.tensor.matmul(
                        psg[:],
                        wh[:, 128 * 3 : 128 * 4],
                        h_t[:],
                        start=False,
                        stop=True,
                    )
                ga = work.tile([H, b_half], F32, tag=f"g{hs}")
                nc.scalar.activation(
                    ga[:], psg[:], AF.Tanh,
                    bias=bg[:] if bg is not None else 0.0,
                )

                ps = psum.tile([H, 3 * b_half], F32, tag=f"ifo{hs % 2}")
                if brow is not None:
                    # K=1 bias matmuls (bias row x ones) for i, f, o.
                    # Emitted first: they have no h dependency, so they fill
                    # PE idle time before h_t is ready.
                    for j in range(3):
                        sl = slice(b_half * j, b_half * (j + 1))
                        nc.tensor.matmul(
                            ps[:, sl],
                            brow[:, 128 * j : 128 * (j + 1)],
                            w_ones[:, hs * b_half : (hs + 1) * b_half],
                            start=True,
                            stop=False,
                        )
                for j in range(3):                     # banks [i, f, o]
                    sl = slice(b_half * j, b_half * (j + 1))
                    nc.tensor.matmul(
                        ps[:, sl],
                        wi[:, 128 * j : 128 * (j + 1)],
                        rhs_in[:],
                        start=brow is None,
                        stop=first,
                    )
                    if not first:
                        nc.tensor.matmul(
                            ps[:, sl],
                            wh[:, 128 * j : 128 * (j + 1)],
                            h_t[:],
                            start=False,
                            stop=True,
                        )
                # one sigmoid over the contiguous [i, f, o] span
                ifo = work.tile([H, 3 * b_half], F32, tag=f"ifo{hs}")
                nc.scalar.activation(ifo[:], ps[:], AF.Sigmoid)
                i_s = ifo[:, 0:b_half]
                f_s = ifo[:, b_half : 2 * b_half]
                o_s = ifo[:, 2 * b_half : 3 * b_half]
                if first:
                    # c = i*g
                    nc.vector.tensor_mul(c_t[:], i_s, ga[:])
                else:
                    # c = f*c + i*g (i*g in place on ga)
                    eng_t1 = nc.gpsimd if t1_gpsimd else nc.vector
                    eng_t1.tensor_mul(ga[:], ga[:], i_s)
                    nc.vector.tensor_mul(c_t[:], c_t[:], f_s)
                    nc.vector.tensor_add(c_t[:], c_t[:], ga[:])
                th = work.tile([H, b_half], F32, tag=f"th{hs}")
                nc.scalar.activation(th[:], c_t[:], AF.Tanh)
                h_new = hpool.tile([H, b_half], F32R, tag=htag)
                nc.vector.tensor_mul(h_new[:], o_s, th[:])
                return h_new

            for _rep in range(n_repeat):
                c0 = [
                    cpool.tile([H, b_half], F32, tag=f"c0_{s}", name=f"c0_{s}")
                    for s in range(n_split)
                ]
                c1 = [
                    cpool.tile([H, b_half], F32, tag=f"c1_{s}", name=f"c1_{s}")
                    for s in range(n_split)
                ]
                h0 = [None] * n_split
                h1 = [None] * n_split

                # ---------------- encoder ----------------
                for t in range(t_in):
                    xts = []
                    for s in range(n_split):
                        xt = xpool.tile([F + 1, b_half], F32R, tag=f"xt{s}")
                        nc.sync.dma_start(
                            xt[:], xT[:, t, b_half * s : b_half * (s + 1)]
                        )
                        xts.append(xt)
                    for s in range(n_split):
                        h0[s] = layer_step(
                            s, w_wih0e, w_whh0e, None, None, xts[s], h0[s],
                            c0[s], f"h0_{s}", first=(t == 0),
                        )
                    for s in range(n_split):
                        h1[s] = layer_step(
                            s, w_wih1e, w_whh1e, w_be1, t_bg_e1, h0[s], h1[s],
                            c1[s], f"h1_{s}", first=(t == 0),
                        )

                # ---------------- decoder ----------------
                for t in range(t_out):
                    for s in range(n_split):
                        if t == 0:
                            xt = xpool.tile([F + 1, b_half], F32R, tag=f"xt{s}")
                            nc.sync.dma_start(
                                xt[:],
                                xT[:, t_in - 1, b_half * s : b_half * (s + 1)],
                            )
                            h0[s] = layer_step(
                                s, w_wih0d, w_whh0d, None, None, xt, h0[s],
                                c0[s], f"h0_{s}",
                            )
                        else:
                            h0[s] = layer_step(
                                s, w_wcomb, w_whh0d, w_bcomb, t_bg_comb,
                                h1[s], h0[s], c0[s], f"h0_{s}",
                            )
                    for s in range(n_split):
                        h1[s] = layer_step(
                            s, w_wih1d, w_whh1d, w_bd1, t_bg_d1, h0[s],
                            h1[s], c1[s], f"h1_{s}",
                        )
                    for s in range(n_split):
                        # pred_t.T per 128-batch tile:
                        # psf[:, 2j:2j+2] = h1_j.T @ fcWT
                        psf = psum.tile([H, b_half], F32, tag=f"gg{s % 2}")
                        for j in range(nj_h):
                            nc.tensor.matmul(
                                psf[:, F * j : F * (j + 1)],
                                h1[s][:, 128 * j : 128 * (j + 1)],
                                w_fcWT[:],
                                start=True,
                                stop=True,
                            )
                        dst = hp3[:, nj_h * s : nj_h * (s + 1), F * t : F * (t + 1)]
                        src = psf[:, : F * nj_h].rearrange(
                            "p (j q) -> p j q", q=F
                        )
                        fcbs = fcb3[:, nj_h * s : nj_h * (s + 1), :]
                        nc.vector.scalar_tensor_tensor(
                            dst, src, 0.0, fcbs, ALU.add, ALU.add
                        )

                # ---------------- output ----------------
                for j in range(nj):
                    nc.sync.dma_start(
                        out[128 * j : 128 * (j + 1), :],
                        hp[:, F * t_out * j : F * t_out * (j + 1)],
                    )

    nc.compile()
    return nc


def host_prep(inputs, b_core=B_CORE, n_cores=N_CORES):
    """Numpy-side packing of inputs into per-core in_maps."""
    f32 = np.float32

    def pack_w(w):  # [4H, K] -> [K, 4H] with gate blocks reordered to GORD
        wT = np.ascontiguousarray(w.T).astype(f32)          # [K, 4H]
        w4 = wT.reshape(-1, 4, H)
        return np.ascontiguousarray(
            np.stack([w4[:, k] for k in GORD], axis=1).reshape(-1, 4 * H)
        )

    def pack_b_row(b):  # [4H] -> [1, 4H] in GORD order
        b4 = b.reshape(4, H)
        return np.concatenate([b4[k] for k in GORD])[None, :].astype(f32)

    def aug(w, b):  # weight + bias row for ones-augmented input
        return np.concatenate([pack_w(w), pack_b_row(b)], axis=0)

    def g_bias(b):  # [4H] -> [H, 1] gate-g slice
        return np.ascontiguousarray(b.reshape(4, H)[2][:, None]).astype(f32)

    fc_W = inputs["fc_W"]
    fc_b = inputs["fc_b"]
    dec_Wih0 = inputs["dec_Wih0"]
    wcomb = dec_Wih0 @ fc_W                     # [4H, H]
    bcomb = inputs["dec_b0"] + dec_Wih0 @ fc_b  # [4H]
    nj = b_core // 128

    shared = {
        "wih0T_e": aug(inputs["enc_Wih0"], inputs["enc_b0"]),
        "whh0T_e": pack_w(inputs["enc_Whh0"]),
        "wih1T_e": pack_w(inputs["enc_Wih1"]),
        "whh1T_e": pack_w(inputs["enc_Whh1"]),
        "wih0T_d": aug(dec_Wih0, inputs["dec_b0"]),
        "wcombT": pack_w(wcomb),
        "whh0T_d": pack_w(inputs["dec_Whh0"]),
        "wih1T_d": pack_w(inputs["dec_Wih1"]),
        "whh1T_d": pack_w(inputs["dec_Whh1"]),
        "brow_e1": pack_b_row(inputs["enc_b1"]),
        "brow_comb": pack_b_row(bcomb),
        "brow_d1": pack_b_row(inputs["dec_b1"]),
        "ones_r": np.ones((1, b_core), f32),
        "bg_e1": g_bias(inputs["enc_b1"]),
        "bg_comb": g_bias(bcomb),
        "bg_d1": g_bias(inputs["dec_b1"]),
        "fcWT": np.ascontiguousarray(fc_W.T).astype(f32),
        "fcb": np.ascontiguousarray(
            np.broadcast_to(np.tile(fc_b.astype(f32), nj), (H, F * nj))
        ),
    }

    x = inputs["x"]
    in_maps = []
    for c in range(n_cores):
        xc = x[c * b_core : (c + 1) * b_core]          # [b, F, T]
        xT = np.ascontiguousarray(xc.transpose(1, 2, 0)).astype(f32)
        xT = np.concatenate([xT, np.ones((1,) + xT.shape[1:], f32)], axis=0)
        in_maps.append({"xT": xT, **shared})
    return in_maps


N_SPLIT = 4

_CACHE = {}


def _get_program(n_repeat=1):
    key = (B_CORE, T_IN, T_OUT, N_SPLIT, n_repeat)
    if key not in _CACHE:
        _CACHE[key] = build_program(B_CORE, T_IN, T_OUT, N_SPLIT, n_repeat)
    return _CACHE[key]


def kernel(**inputs) -> np.ndarray:
    from concourse.bass_utils import run_bass_kernel_spmd

    inputs = {k: np.asarray(v) for k, v in inputs.items()}
    nc = _get_program()
    in_maps = host_prep(inputs)
    res = run_bass_kernel_spmd(
        nc, in_maps, core_ids=list(range(N_CORES)), trace=False
    )
    outs = [
        res.results[c]["out"].reshape(B_CORE, T_OUT, F) for c in range(N_CORES)
    ]
    return np.concatenate(outs, axis=0).astype(np.float32)


# revision 12
# speedup vs baseline: 1.2022x; 1.2022x over previous
"""Trainium2 Bass kernel for a 2-layer encoder/decoder LSTM (CS-LSTM).

Model (hardcoded): B=16384, F=2, T_IN=30, T_OUT=50, H=128.
Sharding: pure data parallel over batch across 8 NeuronCores (B_CORE=2048),
weights replicated. No cross-core communication.

Device layout: "hidden on partitions" — h, c, gate tensors are [H=128, B_CORE]
so every recurrent matmul is out[128, N] = W_T.T @ h with K=128 on partitions.

Host-side preprocessing (numpy, done once inside kernel()):
  - x is transposed to xT[f, t, b] so the per-step input x_t is a cheap
    contiguous [2, B_CORE] DMA.
  - all weights are pre-transposed into matmul lhsT layout [K, 4H].
  - the decoder input projection is composed with the fc layer:
        Wih0_dec @ (fc_W @ h2 + fc_b) = (Wih0_dec@fc_W) @ h2 + Wih0_dec@fc_b
    which removes the fc matmul from the recurrent critical path.
  - per-gate biases packed as [128, 4] tiles (column k = gate k).

The fc output needed for the predictions is computed per decoder step with
h2-tiles as the stationary operand (out = h2_tile.T @ fc_W.T = pred.T), which
lands predictions directly in batch-major layout for a cheap final DMA.
"""

import os

os.environ.setdefault("MYCRO_LOCAL_CACHE", "1")

import numpy as np
from contextlib import ExitStack

import concourse.bass as bass
import concourse.tile as tile
from concourse import bacc, mybir

B, F, T_IN, T_OUT, H = 16384, 2, 30, 50, 128
N_CORES = 8
B_CORE = B // N_CORES

F32 = mybir.dt.float32
F32R = mybir.dt.float32r
BF16 = mybir.dt.bfloat16
AF = mybir.ActivationFunctionType
ALU = mybir.AluOpType

# matmul moving-operand (free dim) max for 4-byte dtypes
MM_N = 512


def build_program(b_core=B_CORE, t_in=T_IN, t_out=T_OUT, n_split=2, n_repeat=1,
                  mm_bf16=False, c_bf16=False, work_bufs=2, h_bufs=3, x_bufs=3,
                  psum_bufs=2, t1_gpsimd=False):
    """Build + compile the single-core program (same program on all cores).

    n_split: number of independent batch slices processed as interleaved
    recurrence chains (more slices -> more instruction-level parallelism
    across engines at the cost of per-instruction overhead).
    n_repeat: repeat the whole computation (timing only).
    mm_bf16: bf16 matmul path (weights, h, x, gate activations) -- FWL weight
    loads and 2x DVE modes; PSUM accumulation stays fp32.
    c_bf16: keep the cell state in bf16 too (fastest DVE, more rounding).
    """
    nc = bacc.Bacc("TRN2", target_bir_lowering=False, debug=False)
    DT_MM = BF16 if mm_bf16 else F32R          # matmul operand dtype
    DT_G = BF16 if mm_bf16 else F32            # gate activation dtype
    DT_C = BF16 if c_bf16 else F32             # cell state dtype

    nj = b_core // 128             # number of 128-wide batch tiles (fc)
    b_half = b_core // n_split     # per-chain batch width
    nj_h = b_half // 128
    mm_n = min(MM_N, b_half)       # matmul N-chunk size
    nch = b_half // mm_n           # number of matmul N-chunks per chain
    assert b_half % mm_n == 0 and b_half % 128 == 0

    def din(name, shape, dt=None):
        return nc.dram_tensor(name, shape, dt or DT_MM, kind="ExternalInput").ap()

    # inputs
    xT = din("xT", [F, t_in, b_core])                 # x[b,f,t] -> [f,t,b]
    wih0T_e = din("wih0T_e", [F, 4 * H])              # enc_Wih0.T
    whh0T_e = din("whh0T_e", [H, 4 * H])              # enc_Whh0.T
    wih1T_e = din("wih1T_e", [H, 4 * H])
    whh1T_e = din("whh1T_e", [H, 4 * H])
    wih0T_d = din("wih0T_d", [F, 4 * H])              # dec step-0 input proj
    wcombT = din("wcombT", [H, 4 * H])                # (dec_Wih0@fc_W).T
    whh0T_d = din("whh0T_d", [H, 4 * H])
    wih1T_d = din("wih1T_d", [H, 4 * H])
    whh1T_d = din("whh1T_d", [H, 4 * H])
    fcWT = din("fcWT", [H, F])                        # fc_W.T
    be0 = din("be0", [H, 4], F32)                     # enc_b0 as [128,4]
    be1 = din("be1", [H, 4], F32)
    bd0 = din("bd0", [H, 4], F32)                     # dec_b0 (step 0 only)
    bcomb = din("bcomb", [H, 4], F32)                 # dec_b0 + dec_Wih0@fc_b
    bd1 = din("bd1", [H, 4], F32)
    fcb = din("fcb", [H, F * nj], F32)                # fc_b tiled nj times

    out = nc.dram_tensor(
        "out", [b_core, F * t_out], F32, kind="ExternalOutput"
    ).ap()

    with tile.TileContext(nc) as tc:
        with ExitStack() as ctx:
            wpool = ctx.enter_context(tc.tile_pool(name="weights", bufs=1))
            hpool = ctx.enter_context(tc.tile_pool(name="hstate", bufs=h_bufs))
            cpool = ctx.enter_context(tc.tile_pool(name="cstate", bufs=1))
            work = ctx.enter_context(tc.tile_pool(name="work", bufs=work_bufs))
            xpool = ctx.enter_context(tc.tile_pool(name="xin", bufs=x_bufs))
            psum = ctx.enter_context(
                tc.tile_pool(name="psum", bufs=psum_bufs, space="PSUM")
            )

            def load_w(ap, dt=None):
                t = wpool.tile(list(ap.shape), dt or DT_MM, tag=f"w_{ap.name}")
                nc.sync.dma_start(t[:], ap[:])
                return t

            w_wih0e = load_w(wih0T_e)
            w_whh0e = load_w(whh0T_e)
            w_wih1e = load_w(wih1T_e)
            w_whh1e = load_w(whh1T_e)
            w_wih0d = load_w(wih0T_d)
            w_wcomb = load_w(wcombT)
            w_whh0d = load_w(whh0T_d)
            w_wih1d = load_w(wih1T_d)
            w_whh1d = load_w(whh1T_d)
            w_fcWT = load_w(fcWT)
            t_be0 = load_w(be0, F32)
            t_be1 = load_w(be1, F32)
            t_bd0 = load_w(bd0, F32)
            t_bcomb = load_w(bcomb, F32)
            t_bd1 = load_w(bd1, F32)
            t_fcb = load_w(fcb, F32)

            # prediction accumulator: [128, nj*F*t_out], batch tile j occupies
            # free range [j*F*t_out, (j+1)*F*t_out), layout (t, f) within.
            hp = wpool.tile([128, nj * F * t_out], F32, tag="hpred")

            hp3 = hp[:].rearrange("p (j q) -> p j q", q=F * t_out)
            fcb3 = t_fcb[:].rearrange("p (j q) -> p j q", q=F)

            # gate order: process i(0), g(2), f(1), o(3) so the cell-state
            # math can start as early as possible.
            def layer_step(hs, wi, wh, bias, rhs_in, h_t, c_t, htag, first=False):
                a = {}
                for k in ((0, 2, 3) if first else (0, 2, 1, 3)):
                    ps = psum.tile([H, b_half], F32, tag=f"gates{hs}")
                    for j in range(nch):
                        sl = slice(mm_n * j, mm_n * (j + 1))
                        nc.tensor.matmul(
                            ps[:, sl],
                            wi[:, 128 * k : 128 * (k + 1)],
                            rhs_in[:, sl],
                            start=True,
                            stop=first,
                        )
                        if not first:
                            nc.tensor.matmul(
                                ps[:, sl],
                                wh[:, 128 * k : 128 * (k + 1)],
                                h_t[:, sl],
                                start=False,
                                stop=True,
                            )
                    act = work.tile([H, b_half], DT_G, tag=f"act{k}_{hs}")
                    nc.scalar.activation(
                        act[:],
                        ps[:],
                        AF.Tanh if k == 2 else AF.Sigmoid,
                        bias=bias[:, k : k + 1],
                    )
                    a[k] = act
                if first:
                    # c = i*g
                    nc.vector.tensor_mul(c_t[:], a[0][:], a[2][:])
                else:
                    # c = f*c + i*g   (in place on c_t; i*g in place on a[0]).
                    # i*g optionally runs on GpSimd to relieve the DVE.
                    eng_t1 = nc.gpsimd if t1_gpsimd else nc.vector
                    eng_t1.tensor_mul(a[0][:], a[0][:], a[2][:])
                    nc.vector.tensor_mul(c_t[:], c_t[:], a[1][:])
                    nc.vector.tensor_add(c_t[:], c_t[:], a[0][:])
                th = work.tile([H, b_half], DT_G, tag=f"th{hs}")
                nc.scalar.activation(th[:], c_t[:], AF.Tanh)
                h_new = hpool.tile([H, b_half], DT_MM, tag=htag)
                nc.vector.tensor_mul(h_new[:], a[3][:], th[:])
                return h_new

            for _rep in range(n_repeat):
                # independent recurrence chains, one per batch slice
                c0 = [
                    cpool.tile([H, b_half], DT_C, tag=f"c0_{s}", name=f"c0_{s}")
                    for s in range(n_split)
                ]
                c1 = [
                    cpool.tile([H, b_half], DT_C, tag=f"c1_{s}", name=f"c1_{s}")
                    for s in range(n_split)
                ]
                h0 = [None] * n_split
                h1 = [None] * n_split

                # ---------------- encoder ----------------
                for t in range(t_in):
                    xts = []
                    for s in range(n_split):
                        xt = xpool.tile([F, b_half], DT_MM, tag=f"xt{s}")
                        nc.sync.dma_start(
                            xt[:], xT[:, t, b_half * s : b_half * (s + 1)]
                        )
                        xts.append(xt)
                    for s in range(n_split):
                        h0[s] = layer_step(
                            s, w_wih0e, w_whh0e, t_be0, xts[s], h0[s], c0[s],
                            f"h0_{s}", first=(t == 0),
                        )
                    for s in range(n_split):
                        h1[s] = layer_step(
                            s, w_wih1e, w_whh1e, t_be1, h0[s], h1[s], c1[s],
                            f"h1_{s}", first=(t == 0),
                        )

                # ---------------- decoder ----------------
                for t in range(t_out):
                    for s in range(n_split):
                        if t == 0:
                            xt = xpool.tile([F, b_half], DT_MM, tag=f"xt{s}")
                            nc.sync.dma_start(
                                xt[:],
                                xT[:, t_in - 1, b_half * s : b_half * (s + 1)],
                            )
                            h0[s] = layer_step(
                                s, w_wih0d, w_whh0d, t_bd0, xt, h0[s], c0[s],
                                f"h0_{s}",
                            )
                        else:
                            h0[s] = layer_step(
                                s, w_wcomb, w_whh0d, t_bcomb, h1[s], h0[s],
                                c0[s], f"h0_{s}",
                            )
                    for s in range(n_split):
                        h1[s] = layer_step(
                            s, w_wih1d, w_whh1d, t_bd1, h0[s], h1[s], c1[s],
                            f"h1_{s}",
                        )
                    for s in range(n_split):
                        # pred_t.T per 128-batch tile:
                        # psum[:, 2j:2j+2] = h1_j.T @ fcWT
                        psf = psum.tile([H, b_half], F32, tag=f"gates{s}")
                        for j in range(nj_h):
                            nc.tensor.matmul(
                                psf[:, F * j : F * (j + 1)],
                                h1[s][:, 128 * j : 128 * (j + 1)],
                                w_fcWT[:],
                                start=True,
                                stop=True,
                            )
                        # H_pred[:, j_global, 2t:2t+2] = psf[:, 2j:2j+2] + fc_b
                        dst = hp3[:, nj_h * s : nj_h * (s + 1), F * t : F * (t + 1)]
                        src = psf[:, : F * nj_h].rearrange(
                            "p (j q) -> p j q", q=F
                        )
                        fcbs = fcb3[:, nj_h * s : nj_h * (s + 1), :]
                        nc.vector.scalar_tensor_tensor(
                            dst, src, 0.0, fcbs, ALU.add, ALU.add
                        )

                # ---------------- output ----------------
                for j in range(nj):
                    nc.sync.dma_start(
                        out[128 * j : 128 * (j + 1), :],
                        hp[:, F * t_out * j : F * t_out * (j + 1)],
                    )

    nc.compile()
    return nc


def host_prep(inputs, b_core=B_CORE, n_cores=N_CORES, mm_bf16=False):
    """Numpy-side packing of inputs into per-core in_maps."""
    f32 = np.float32
    if mm_bf16:
        import ml_dtypes
        mmdt = ml_dtypes.bfloat16
    else:
        mmdt = np.float32

    def gates_T(w):  # [4H, K] -> [K, 4H]
        return np.ascontiguousarray(w.T).astype(mmdt)

    def bias_tile(b):  # [4H] -> [H, 4]
        return np.ascontiguousarray(b.reshape(4, H).T).astype(f32)

    fc_W = inputs["fc_W"]
    fc_b = inputs["fc_b"]
    dec_Wih0 = inputs["dec_Wih0"]
    wcomb = dec_Wih0 @ fc_W                     # [4H, H]
    bcomb = inputs["dec_b0"] + dec_Wih0 @ fc_b  # [4H]
    nj = b_core // 128

    shared = {
        "wih0T_e": gates_T(inputs["enc_Wih0"]),
        "whh0T_e": gates_T(inputs["enc_Whh0"]),
        "wih1T_e": gates_T(inputs["enc_Wih1"]),
        "whh1T_e": gates_T(inputs["enc_Whh1"]),
        "wih0T_d": gates_T(dec_Wih0),
        "wcombT": gates_T(wcomb),
        "whh0T_d": gates_T(inputs["dec_Whh0"]),
        "wih1T_d": gates_T(inputs["dec_Wih1"]),
        "whh1T_d": gates_T(inputs["dec_Whh1"]),
        "fcWT": np.ascontiguousarray(fc_W.T).astype(mmdt),
        "be0": bias_tile(inputs["enc_b0"]),
        "be1": bias_tile(inputs["enc_b1"]),
        "bd0": bias_tile(inputs["dec_b0"]),
        "bcomb": bias_tile(bcomb),
        "bd1": bias_tile(inputs["dec_b1"]),
        "fcb": np.ascontiguousarray(
            np.broadcast_to(np.tile(fc_b.astype(f32), nj), (H, F * nj))
        ),
    }

    x = inputs["x"]
    in_maps = []
    for c in range(n_cores):
        xc = x[c * b_core : (c + 1) * b_core]          # [b, F, T]
        xT = np.ascontiguousarray(xc.transpose(1, 2, 0)).astype(mmdt)
        in_maps.append({"xT": xT, **shared})
    return in_maps


# shipped configuration (n_split=4 measured fastest on HW: 2.07 ms vs
# 2.22 ms at n_split=2 -- real chains stall more than the cost model thinks)
N_SPLIT = 4
MM_BF16 = False
C_BF16 = False

_CACHE = {}


def _get_program(n_repeat=1):
    key = (B_CORE, T_IN, T_OUT, N_SPLIT, n_repeat, MM_BF16, C_BF16)
    if key not in _CACHE:
        _CACHE[key] = build_program(*key)
    return _CACHE[key]


def kernel(**inputs) -> np.ndarray:
    from concourse.bass_utils import run_bass_kernel_spmd

    inputs = {k: np.asarray(v) for k, v in inputs.items()}
    nc = _get_program()
    in_maps = host_prep(inputs, mm_bf16=MM_BF16)
    res = run_bass_kernel_spmd(
        nc, in_maps, core_ids=list(range(N_CORES)), trace=False
    )
    outs = [
        res.results[c]["out"].reshape(B_CORE, T_OUT, F) for c in range(N_CORES)
    ]
    return np.concatenate(outs, axis=0).astype(np.float32)

